# revision 6
# baseline (speedup 1.0000x reference)
"""NeuralODE forward (Euler, whole-sequence) on 8 Trainium2 NeuronCores.

Math (per step): z <- z + h * (tanh([z, u] @ W1 + b1) @ W2 + b2)
Shapes: z0 [4096, 256], u [4096, 64], W1 [320, 1024], W2 [1024, 256],
t [11] uniform grid -> 10 Euler steps of h = 0.05.

Sharding: data-parallel over the batch axis. Each core owns 512 rows,
weights replicated, no cross-core communication.

Device layout: everything is kept transposed (features on partitions,
batch on the free axis) so both matmuls run with batch as the moving
free dim:
    hT = tanh(W1z.T @ zT + c_ub)        [1024, 512]
    zT <- zT + (W2.T @ hT + b2) * h     [256, 512]
where c_ub = W1u.T @ uT + b1 is hoisted out of the step loop (u is
constant in time), removing the K=64 remainder chunk from every step.
Matmul operands are bitcast to float32r for the full-rate fp32 PE path.
"""

import math
import sys

import numpy as np

sys.path.insert(0, "/opt/trn_rl_repo")

import concourse.mybir as mybir
import concourse.tile as tile
from concourse import bacc
from concourse.bass import ts
from concourse.bass_utils import run_bass_kernel_spmd

H_MAX = 0.05
N_CORES = 8
P = 128

B = 512  # batch rows per core
D = 256  # z dim -> 2 partition chunks
U = 64  # u dim
H = 1024  # hidden -> 8 partition chunks
KD = D // P  # 2
KH = H // P  # 8

USE_F32R = True  # full-rate fp32 matmul path
TRACE = False  # set by test.py to collect a HW profile
TRACE_DIR = None  # set by test.py: directory for NTFF/perfetto artifacts

_program_cache: dict = {}


def _steps_from_t(t_np):
    """Replicate the reference's trace-time step derivation.

    Returns a list of (h, emit) pairs: one entry per Euler micro-step;
    emit=True on the last micro-step of each grid interval (that state
    is recorded into the output sequence).
    """
    steps = []
    for i_t in range(t_np.shape[0] - 1):
        t0f, t1f = float(t_np[i_t]), float(t_np[i_t + 1])
        n_steps = int(math.ceil(abs(t1f - t0f) / H_MAX))
        h = np.float32((t1f - t0f) / n_steps)
        for s in range(n_steps):
            steps.append((float(h), s == n_steps - 1))
    return steps


def _build_program(steps):
    f32 = mybir.dt.float32
    f32r = mybir.dt.float32r

    def mm(ap):
        return ap

    nc = bacc.Bacc(
        "TRN2", target_bir_lowering=False, debug=False, num_devices=N_CORES
    )

    z0t = nc.dram_tensor("z0t", [D, B], f32r, kind="ExternalInput")
    ut = nc.dram_tensor("ut", [U, B], f32r, kind="ExternalInput")
    w1r = nc.dram_tensor("w1r", [P, KD, H], f32r, kind="ExternalInput")
    w1u = nc.dram_tensor("w1u", [U, H], f32r, kind="ExternalInput")
    w2r = nc.dram_tensor("w2r", [P, KH, D], f32r, kind="ExternalInput")
    b1t = nc.dram_tensor("b1t", [P, KH], f32, kind="ExternalInput")
    b2t = nc.dram_tensor("b2t", [P, KD], f32, kind="ExternalInput")
    n_rec = sum(1 for _, e in steps if e)
    out = nc.dram_tensor("out", [n_rec, D, B], f32r, kind="ExternalOutput")

    Tanh = mybir.ActivationFunctionType.Tanh
    add = mybir.AluOpType.add
    mult = mybir.AluOpType.mult

    with tile.TileContext(nc) as tc:
        with (
            tc.tile_pool(name="const", bufs=1) as const,
            tc.tile_pool(name="zpool", bufs=3) as zpool,
            tc.tile_pool(name="hpool", bufs=2) as hpool,
            tc.tile_pool(name="tmp", bufs=3) as tmp,
            tc.tile_pool(name="psum", bufs=3, space="PSUM") as psum,
        ):
            w1_sb = const.tile([P, KD, H], f32r)
            nc.sync.dma_start(out=w1_sb[:], in_=w1r[:])
            w1u_sb = const.tile([U, H], f32r)
            nc.sync.dma_start(out=w1u_sb[:], in_=w1u[:])
            w2_sb = const.tile([P, KH, D], f32r)
            nc.sync.dma_start(out=w2_sb[:], in_=w2r[:])
            b1_sb = const.tile([P, KH], f32)
            nc.sync.dma_start(out=b1_sb[:], in_=b1t[:])
            b2_sb = const.tile([P, KD], f32)
            nc.sync.dma_start(out=b2_sb[:], in_=b2t[:])
            ut_sb = const.tile([U, B], f32r)
            nc.sync.dma_start(out=ut_sb[:], in_=ut[:])

            # c_ub[m] = W1u.T @ uT + b1[m], hoisted out of the step loop
            cub_sb = const.tile([P, KH, B], f32)
            for m in range(KH):
                ps = psum.tile([P, B], f32, tag="ps1")
                nc.tensor.matmul(
                    ps, mm(w1u_sb[:, ts(m, P)]), mm(ut_sb[:]), start=True, stop=True
                )
                nc.vector.tensor_scalar(
                    out=cub_sb[:, m, :],
                    in0=ps,
                    scalar1=b1_sb[:, m : m + 1],
                    scalar2=None,
                    op0=add,
                )

            z_cur = []
            for n in range(KD):
                zt = zpool.tile([P, B], f32r, tag=f"z{n}")
                nc.sync.dma_start(out=zt[:], in_=z0t[ts(n, P), :])
                z_cur.append(zt)

            rec = 0
            for h_i, emit in steps:
                h_tiles = []
                for m in range(KH):
                    ps1 = psum.tile([P, B], f32, tag="ps1")
                    for n in range(KD):
                        nc.tensor.matmul(
                            ps1,
                            mm(w1_sb[:, n, ts(m, P)]),
                            mm(z_cur[n][:]),
                            start=(n == 0),
                            stop=(n == KD - 1),
                        )
                    tadd = tmp.tile([P, B], f32, tag="tadd")
                    nc.vector.tensor_add(tadd, ps1, cub_sb[:, m, :])
                    ht = hpool.tile([P, B], f32r, tag=f"h{m}")
                    nc.scalar.activation(ht, tadd, Tanh)
                    h_tiles.append(ht)

                z_new = []
                for n in range(KD):
                    ps2 = psum.tile([P, B], f32, tag="ps2")
                    for k in range(KH):
                        nc.tensor.matmul(
                            ps2,
                            mm(w2_sb[:, k, ts(n, P)]),
                            mm(h_tiles[k][:]),
                            start=(k == 0),
                            stop=(k == KH - 1),
                        )
                    delta = tmp.tile([P, B], f32, tag="delta")
                    nc.vector.tensor_scalar(
                        out=delta,
                        in0=ps2,
                        scalar1=b2_sb[:, n : n + 1],
                        scalar2=float(h_i),
                        op0=add,
                        op1=mult,
                    )
                    znew = zpool.tile([P, B], f32r, tag=f"z{n}")
                    nc.vector.tensor_add(znew, z_cur[n], delta)
                    if emit:
                        nc.sync.dma_start(out=out[rec, ts(n, P), :], in_=znew[:])
                    z_new.append(znew)
                if emit:
                    rec += 1
                z_cur = z_new

    nc.compile()
    return nc


def kernel(z0, u, t, W1, b1, W2, b2):
    z0 = np.ascontiguousarray(np.asarray(z0, dtype=np.float32))
    u = np.ascontiguousarray(np.asarray(u, dtype=np.float32))
    t_np = np.asarray(t, dtype=np.float32)
    W1 = np.ascontiguousarray(np.asarray(W1, dtype=np.float32))
    b1 = np.ascontiguousarray(np.asarray(b1, dtype=np.float32))
    W2 = np.ascontiguousarray(np.asarray(W2, dtype=np.float32))
    b2 = np.ascontiguousarray(np.asarray(b2, dtype=np.float32))

    bs, dim = z0.shape
    assert (bs, dim) == (N_CORES * B, D), (bs, dim)
    assert u.shape == (bs, U) and W1.shape == (D + U, H)
    assert W2.shape == (H, D) and b1.shape == (H,) and b2.shape == (D,)

    steps = _steps_from_t(t_np)
    n_rec = sum(1 for _, e in steps if e)
    if n_rec == 0:
        return z0[None].copy()

    key = tuple(steps)
    nc = _program_cache.get(key)
    if nc is None:
        nc = _build_program(steps)
        _program_cache[key] = nc

    w1r = np.ascontiguousarray(W1[:D].reshape(KD, P, H).transpose(1, 0, 2))
    w1u = np.ascontiguousarray(W1[D:])
    w2r = np.ascontiguousarray(W2.reshape(KH, P, D).transpose(1, 0, 2))
    b1t = np.ascontiguousarray(b1.reshape(KH, P).T)
    b2t = np.ascontiguousarray(b2.reshape(KD, P).T)

    in_maps = []
    for c in range(N_CORES):
        sl = slice(c * B, (c + 1) * B)
        in_maps.append(
            {
                "z0t": np.ascontiguousarray(z0[sl].T),
                "ut": np.ascontiguousarray(u[sl].T),
                "w1r": w1r,
                "w1u": w1u,
                "w2r": w2r,
                "b1t": b1t,
                "b2t": b2t,
            }
        )

    res = run_bass_kernel_spmd(
        nc, in_maps, list(range(N_CORES)), trace=TRACE, tmpdir=TRACE_DIR
    )
    kernel.last_results = res

    full = np.empty((n_rec + 1, bs, dim), dtype=np.float32)
    full[0] = z0
    for c in range(N_CORES):
        o = res.results[c]["out"]  # [n_rec, D, B] transposed states
        full[1:, c * B : (c + 1) * B, :] = o.transpose(0, 2, 1)
    return full
